# revision 38
# baseline (speedup 1.0000x reference)
"""Causal full-d_model attention (B=4, T=2048, C=1024) on 8 Trainium2 cores.

Sharding: core = 2*b + p handles batch b and two 512-row sequence blocks,
p=0 -> blocks {0, 3}, p=1 -> blocks {1, 2} (pairing balances causal work).
The V projection is split across the pair: each core projects its own
1024-token half (per-core input data places the right half in the same
program slot), then an intra-pair AllGather ([[0,1],[2,3],[4,5],[6,7]] -
the valid LNC1_4x2 shape) assembles the full v.  V is computed FIRST so
the AllGather launches at ~60us and lands long before PV consumes v at
~165us - the collective stream's high variance (35-185 GB/s observed
run-to-run) never touches the critical path.  K is computed in full on
both cores: scores need kT early (~130us), which a collective cannot
reliably meet (entry barrier ~35-45us + ~12us trigger latency +
mood-dependent rate); the redundant K projection costs a predictable
27.6us of matmul instead.

All matmul operands are bf16 (same PE rate as f32r, but FWL halves
LDWEIGHTS and every DMA byte count halves).  Accumulation is fp32 in
PSUM; softmax denominator / reciprocal / biases stay fp32.

On-device layout is transposed ([feature, token]) so every matmul
contracts along the partition axis:
    kT/qT = W.T @ xT            (projection)
    v     = xv.T @ Wv           (token-partition layout)
    scoresT[j, i] = kT_j.T @ qT (j on partitions)
    attnT[c, i]  += v_j.T @ probsT
    outT          = Wo_slice.T @ attnT
Softmax is unnormalized (scores ~ N(0,1), exp is safe); the denominator
comes from an M=1 ones-column matmul over masked exp tiles and is applied
at the PSUM->SBUF copy via a DRAM-broadcast reciprocal row.  Causal masks
arrive as per-core input data, so all 8 cores run one SPMD program.

Input loads ride the sync HWDGE FIFO in exact need-order (~0.6us of
sequencer dispatch per DMA, so tiny transfers are packed into single
tensors); V staging also goes on sync so it drains independently of the
collective trigger queued on gpsimd.  Phase 2 runs both score passes
before any PV (more cover for the v readback), and PV-B group 1 sits
between PV-A and O-A so each slot's attn-multiply latency hides under
the other slot's PE work; PV accumulates in 4-bank PSUM groups.
"""

import math

import numpy as np

P = 128          # SBUF partitions
B_, T_, C_ = 4, 2048, 1024
RG = [[0, 1], [2, 3], [4, 5], [6, 7]]   # intra-pair replica groups


def _emit(nc, tc, aps, T, C):
    import concourse.bass as bass
    from concourse import mybir
    from concourse.tile_rust import add_dep_helper
    from contextlib import ExitStack

    AFT = mybir.ActivationFunctionType
    f32 = mybir.dt.float32
    bf16 = mybir.dt.bfloat16

    NT = C // P            # feature tiles (8)
    BLK = T // 4           # sequence block = i-slot width FB (512)
    TL = 2 * BLK           # local query tokens per core
    TH = T // 2            # V tokens projected per core
    FB = BLK               # matmul moving free dim
    NCHK = T // BLK        # K chunks (4, full sequence)
    NCHL = TH // BLK       # local V chunks (2)
    njA = (2 * BLK) // P   # 8  j-tiles for slot A
    njB = (4 * BLK) // P   # 16 j-tiles for slot B
    CA = P * (njA - 1)
    NJB0 = njA             # slot-B j-tiles < NJB0 are causally all-ones
    CB = P * (njB - 1 - NJB0)
    SC = 1.0 / math.sqrt(C)

    (xT, xv, xq, Wk, Wq, Wv, Wo, bias_t, masks_t, rec_dram, outT) = aps

    with ExitStack() as ctx:
        singles = ctx.enter_context(tc.tile_pool(name="singles", bufs=1))
        kpool = ctx.enter_context(tc.tile_pool(name="kpool", bufs=1))
        qpool = ctx.enter_context(tc.tile_pool(name="qpool", bufs=1))
        vpool = ctx.enter_context(tc.tile_pool(name="vpool", bufs=1))
        wopool = ctx.enter_context(tc.tile_pool(name="wopool", bufs=1))
        psp = ctx.enter_context(tc.tile_pool(name="psp", bufs=8, space="PSUM"))
        dramp = ctx.enter_context(tc.tile_pool(name="dramp", bufs=1,
                                               space="DRAM"))

        # bias3 packs bq/bk/bo; m_all packs ones + both mask masters, so
        # the startup FIFO spends its ~0.6us-per-DMA dispatch budget on
        # the critical loads instead of four tiny transfers.
        bias3 = singles.tile([P, 3, NT], f32, name="bias3")
        m_all = singles.tile([P, 1 + (CA + FB) + (CB + FB)], bf16,
                             name="m_all")
        bq_sb, bk_sb, bo_sb = bias3[:, 0], bias3[:, 1], bias3[:, 2]
        ones_sb = m_all[:, 0:1]
        mA_sb = m_all[:, 1:1 + CA + FB]
        mB_sb = m_all[:, 1 + CA + FB:]

        kT_sb = kpool.tile([P, NT, T], bf16, name="kT_sb")
        qT_sb = qpool.tile([P, NT, TL], bf16, name="qT_sb")
        v_sb = vpool.tile([P, T // P, C], bf16, name="v_sb")
        wo_sb = wopool.tile([P, NT, NT, P], bf16, name="wo_sb")

        # DRAM bounce buffers for the intra-pair V AllGather
        vag_in = dramp.tile([P, TH // P, C], bf16, name="vag_in")
        vag_out = dramp.tile([2, P, TH // P, C], bf16, name="vag_out")

        # ------- phase 1: V-half (+AllGather), K full, then Q -------
        with ExitStack() as p1:
            # wv/wk/wq rotate through 2 slots: wv is dead once the V pass
            # ends (~40us) and wq is not needed until Q (~110us), so wq
            # safely reuses wv's buffer - frees 16KB for 4 resident x chunks
            wpool = p1.enter_context(tc.tile_pool(name="wpool", bufs=2))
            xcpool = p1.enter_context(tc.tile_pool(name="xcpool", bufs=4))
            xvpool = p1.enter_context(tc.tile_pool(name="xvpool", bufs=1))
            xqpool = p1.enter_context(tc.tile_pool(name="xqpool", bufs=1))
            vstp = p1.enter_context(tc.tile_pool(name="vstp", bufs=5))

            wv_sb = wpool.tile([P, 2, NT, C // 2], bf16, name="wv_sb",
                               tag="w")
            wk_sb = wpool.tile([P, NT, NT, P], bf16, name="wk_sb", tag="w")
            wq_sb = wpool.tile([P, NT, NT, P], bf16, name="wq_sb", tag="w")
            xv_sb = xvpool.tile([P, NCHL, NT, BLK], bf16, name="xv_sb")
            xq_sb = xqpool.tile([P, 2, NT, FB], bf16, name="xq_sb")

            # input loads on the sync FIFO in exact need-order
            nc.sync.dma_start(out=xv_sb[:, 0, :NT // 2],
                              in_=xv[:, 0, :NT // 2])
            nc.sync.dma_start(out=xv_sb[:, 0, NT // 2:],
                              in_=xv[:, 0, NT // 2:])
            nc.sync.dma_start(out=wv_sb[:, 0], in_=Wv[:, 0])
            nc.sync.dma_start(out=wv_sb[:, 1], in_=Wv[:, 1])
            nc.sync.dma_start(out=xv_sb[:, 1, :NT // 2],
                              in_=xv[:, 1, :NT // 2])
            nc.sync.dma_start(out=xv_sb[:, 1, NT // 2:],
                              in_=xv[:, 1, NT // 2:])
            nc.sync.dma_start(out=bias3, in_=bias_t)
            for co in range(NT):
                nc.sync.dma_start(out=wk_sb[:, co], in_=Wk[:, co])
            # all four K x chunks load ahead of the V staging DMAs so K
            # never waits on DMAs contending with the running AllGather
            xcs = []
            for jc in range(NCHK):
                xc = xcpool.tile([P, NT, BLK], bf16, name="xc", tag="xc")
                nc.sync.dma_start(out=xc[:, :NT // 2], in_=xT[:, jc, :NT // 2])
                nc.sync.dma_start(out=xc[:, NT // 2:], in_=xT[:, jc, NT // 2:])
                xcs.append(xc)

            # V-half first: v_loc = x_half @ Wv, staged to DRAM per
            # token-tile, so the AllGather triggers at ~45us
            # (bv is folded into bo_t)
            for l in range(NCHL):
                for jt in range(BLK // P):
                    vs = vstp.tile([P, C], bf16, name="vs", tag="vs")
                    for ch in range(2):
                        ps = psp.tile([P, 512], f32, name="ps_v", tag="ps")
                        for ci in range(NT):
                            nc.tensor.matmul(
                                ps,
                                xv_sb[:, l, ci, jt * P:(jt + 1) * P],
                                wv_sb[:, ch, ci, :],
                                start=(ci == 0),
                                stop=(ci == NT - 1),
                            )
                        nc.vector.tensor_copy(
                            vs[:, ch * 512:(ch + 1) * 512], ps
                        )
                    # on sync: drains independently of the collective
                    # trigger queued on gpsimd, so casts never starve
                    nc.sync.dma_start(
                        out=vag_in[:, l * (BLK // P) + jt, :], in_=vs
                    )
            nc.gpsimd.collective_compute(
                "AllGather",
                mybir.AluOpType.bypass,
                replica_groups=RG,
                ins=[vag_in.opt()],
                outs=[vag_out.opt()],
            )

            # remaining input loads, still in need-order
            for co in range(NT):
                nc.sync.dma_start(out=wq_sb[:, co], in_=Wq[:, co])
            nc.sync.dma_start(out=xq_sb, in_=xq)
            nc.sync.dma_start(out=m_all, in_=masks_t)
            nc.sync.dma_start(out=wo_sb, in_=Wo)
            for jc in range(NCHK):
                xc = xcs[jc]
                # K full: kT = Wk.T @ x (+bk), resident in SBUF
                for co in range(NT):
                    ps = psp.tile([P, BLK], f32, name="ps_k", tag="ps")
                    for ci in range(NT):
                        nc.tensor.matmul(
                            ps,
                            wk_sb[:, co, ci, :],
                            xc[:, ci, :],
                            start=(ci == 0),
                            stop=(ci == NT - 1),
                        )
                    nc.scalar.activation(
                        out=kT_sb[:, co, jc * BLK:(jc + 1) * BLK],
                        in_=ps,
                        func=AFT.Identity,
                        bias=bk_sb[:, co:co + 1],
                    )

            # v readback in global token order (h = rank in pair)
            for h in range(2):
                nc.sync.dma_start(
                    out=v_sb[:, h * (TH // P):(h + 1) * (TH // P), :],
                    in_=vag_out[h],
                )

            # Q: qT = Wq.T @ xq (+bq) for the two local 512-blocks
            for s in range(2):
                for co in range(NT):
                    ps = psp.tile([P, FB], f32, name="ps_q", tag="ps")
                    for ci in range(NT):
                        nc.tensor.matmul(
                            ps,
                            wq_sb[:, co, ci, :],
                            xq_sb[:, s, ci, :],
                            start=(ci == 0),
                            stop=(ci == NT - 1),
                        )
                    nc.scalar.activation(
                        out=qT_sb[:, co, s * FB:(s + 1) * FB],
                        in_=ps,
                        func=AFT.Identity,
                        bias=bq_sb[:, co:co + 1],
                    )

        # -------- phase 2: attention + output projection --------
        with ExitStack() as p2:
            probsp = p2.enter_context(tc.tile_pool(name="probsp",
                                                   bufs=njA + njB))
            attnp = p2.enter_context(tc.tile_pool(name="attnp", bufs=2))
            recp = p2.enter_context(tc.tile_pool(name="recp", bufs=2))
            ostagep = p2.enter_context(tc.tile_pool(name="ostagep", bufs=2))

            SLOTS = [(njA, 0, CA, mA_sb), (njB, NJB0, CB, mB_sb)]

            def emit_scores(a):
                nj, j0m, Cm, m_sb = SLOTS[a]
                pjs = []
                ps_den = psp.tile([1, FB], f32, name="ps_den", tag="ps")
                for jt in range(nj):
                    ps_s = psp.tile([P, FB], f32, name="ps_s", tag="ps")
                    for ci in range(NT):
                        nc.tensor.matmul(
                            ps_s,
                            kT_sb[:, ci, jt * P:(jt + 1) * P],
                            qT_sb[:, ci, a * FB:(a + 1) * FB],
                            start=(ci == 0),
                            stop=(ci == NT - 1),
                        )
                    pj = probsp.tile([P, FB], bf16, name="pj", tag="pj")
                    nc.scalar.activation(out=pj, in_=ps_s, func=AFT.Exp, scale=SC)
                    if jt >= j0m:  # earlier j-tiles are all-ones on every core
                        s0 = Cm - P * (jt - j0m)
                        nc.vector.tensor_mul(pj, pj, m_sb[:, s0:s0 + FB])
                    nc.tensor.matmul(
                        ps_den,
                        ones_sb,
                        pj,
                        start=(jt == 0),
                        stop=(jt == nj - 1),
                        skip_group_check=True,
                    )
                    pjs.append(pj)
                # 1/denominator: quick copy releases the PSUM bank, then the
                # slow reciprocal runs off the SBUF copy; broadcast to 128
                # partitions via a stride-0 DRAM read.
                den_sb = recp.tile([1, FB], f32, name="den_sb", tag="den_sb")
                nc.scalar.copy(den_sb, ps_den)
                rrow = recp.tile([1, FB], f32, name="rrow", tag="rrow")
                nc.vector.reciprocal(rrow, den_sb)
                rec_w = nc.sync.dma_start(out=rec_dram[a:a + 1, :], in_=rrow)
                recipB = recp.tile([P, FB], f32, name="recipB", tag="recipB")
                rec_row = rec_dram[a, :]
                rec_bcast = bass.AP(
                    tensor=rec_row.tensor,
                    offset=rec_row.offset,
                    ap=[[0, P]] + [list(d) for d in rec_row.ap],
                )
                rec_r = nc.sync.dma_start(out=recipB, in_=rec_bcast)
                add_dep_helper(rec_r.ins, rec_w.ins, reason="rec_dram RAW")
                return pjs, recipB

            def emit_pv_group(a, pjs, recipB, attn_sb, g0):
                # one 4-bank PSUM accumulation group of PV; the attn
                # multiplies drain the banks while later PE work runs
                nj = SLOTS[a][0]
                ps_attn = [
                    psp.tile([P, FB], f32, name="ps_attn", tag="ps")
                    for _ in range(NT // 2)
                ]
                for jt in range(nj):
                    for k, ct in enumerate(range(g0, g0 + NT // 2)):
                        nc.tensor.matmul(
                            ps_attn[k],
                            v_sb[:, jt, ct * P:(ct + 1) * P],
                            pjs[jt],
                            start=(jt == 0),
                            stop=(jt == nj - 1),
                            skip_group_check=True,
                        )
                for k, ct in enumerate(range(g0, g0 + NT // 2)):
                    nc.vector.tensor_mul(attn_sb[:, ct, :], ps_attn[k], recipB)

            def emit_oproj(a, attn_sb):
                for co in range(NT):
                    ps_o = psp.tile([P, FB], f32, name="ps_o", tag="ps")
                    for ci in range(NT):
                        nc.tensor.matmul(
                            ps_o,
                            wo_sb[:, co, ci, :],
                            attn_sb[:, ci, :],
                            start=(ci == 0),
                            stop=(ci == NT - 1),
                        )
                    os_ = ostagep.tile([P, FB], f32, name="os_", tag="os")
                    nc.scalar.activation(
                        out=os_, in_=ps_o, func=AFT.Identity,
                        bias=bo_sb[:, co:co + 1],
                    )
                    nc.sync.dma_start(
                        out=outT[co * P:(co + 1) * P, a * FB:(a + 1) * FB],
                        in_=os_,
                    )

            # Both score passes run before any PV (extra cover for the v
            # readback); PV-B group 1 sits between PV-A and O-A so the
            # attn-multiply latency hides under the other slot's PE work.
            pjs_A, recB_A = emit_scores(0)
            pjs_B, recB_B = emit_scores(1)
            attn_A = attnp.tile([P, NT, FB], bf16, name="attn_A", tag="attn")
            attn_B = attnp.tile([P, NT, FB], bf16, name="attn_B", tag="attn")
            emit_pv_group(0, pjs_A, recB_A, attn_A, 0)
            emit_pv_group(0, pjs_A, recB_A, attn_A, NT // 2)
            emit_pv_group(1, pjs_B, recB_B, attn_B, 0)
            emit_oproj(0, attn_A)
            emit_pv_group(1, pjs_B, recB_B, attn_B, NT // 2)
            emit_oproj(1, attn_B)


def build_program(T=T_, C=C_, num_cores=8):
    """Build and compile the SPMD Bass program."""
    from concourse import bacc, mybir
    import concourse.tile as tile

    f32 = mybir.dt.float32
    bf16 = mybir.dt.bfloat16
    NT = C // P
    BLK = T // 4
    TL = 2 * BLK
    njA = (2 * BLK) // P
    njB = (4 * BLK) // P
    CA = P * (njA - 1)
    CB = P * (njB - 1 - njA)

    nc = bacc.Bacc(
        "TRN2", target_bir_lowering=False, debug=False, num_devices=num_cores
    )
    xT = nc.dram_tensor("xT", [P, 4, NT, BLK], bf16, kind="ExternalInput").ap()
    xv = nc.dram_tensor("xv", [P, 2, NT, BLK], bf16, kind="ExternalInput").ap()
    xq = nc.dram_tensor("xq", [P, 2, NT, BLK], bf16, kind="ExternalInput").ap()
    Wk = nc.dram_tensor("Wk", [P, NT, NT, P], bf16, kind="ExternalInput").ap()
    Wq = nc.dram_tensor("Wq", [P, NT, NT, P], bf16, kind="ExternalInput").ap()
    Wv = nc.dram_tensor("Wv", [P, 2, NT, C // 2], bf16,
                        kind="ExternalInput").ap()
    Wo = nc.dram_tensor("Wo", [P, NT, NT, P], bf16, kind="ExternalInput").ap()
    bias_t = nc.dram_tensor("bias_t", [P, 3, NT], f32,
                            kind="ExternalInput").ap()
    masks_t = nc.dram_tensor(
        "masks_t", [P, 1 + (CA + BLK) + (CB + BLK)], bf16,
        kind="ExternalInput",
    ).ap()
    rec_dram = nc.dram_tensor("rec_int", [2, BLK], f32).ap()
    outT = nc.dram_tensor("outT", [C, TL], f32, kind="ExternalOutput").ap()

    aps = (xT, xv, xq, Wk, Wq, Wv, Wo, bias_t, masks_t, rec_dram, outT)
    with tile.TileContext(nc) as tc:
        _emit(nc, tc, aps, T, C)
    nc.compile()
    return nc


def make_core_inputs(x, Wq, bq, Wk, bk, Wv, bv, Wo, bo, T=T_, C=C_):
    """Per-core input maps (list of 8 dicts) for the SPMD program."""
    import ml_dtypes

    f = np.float32
    bf = ml_dtypes.bfloat16
    NT = C // P
    BLK = T // 4
    njA = (2 * BLK) // P
    njB = (4 * BLK) // P
    CA = P * (njA - 1)
    CB = P * (njB - 1 - njA)

    x = np.asarray(x, f)
    Wq, Wk, Wv, Wo = (np.asarray(w, f) for w in (Wq, Wk, Wv, Wo))
    bq, bk, bv, bo = (np.asarray(b, f) for b in (bq, bk, bv, bo))

    def panels(W):  # [C, C] -> [P, co, ci, m]: W[ci*P+p, co*P+m]
        return np.ascontiguousarray(
            W.reshape(NT, P, NT, P).transpose(1, 2, 0, 3)
        ).astype(bf)

    Wk_t = panels(Wk)
    Wq_t = panels(Wq)
    Wo_t = panels(Wo)
    # [C, C] -> [P, ch, ci, m']: Wv[ci*P+p, ch*(C/2)+m']  (ch-major halves)
    Wv_t = np.ascontiguousarray(
        Wv.reshape(NT, P, 2, C // 2).transpose(1, 2, 0, 3)
    ).astype(bf)
    bo_eff = (bv @ Wo + bo).astype(f)

    def tr(b):  # [C] -> [P, NT] with b_t[p, t] = b[t*P + p]
        return np.ascontiguousarray(b.reshape(NT, P).T)

    def mask(CC, i0, width):
        pp = np.arange(P, dtype=np.int64)[:, None]
        gg = np.arange(width, dtype=np.int64)[None, :]
        return (pp <= gg - CC + i0).astype(bf)

    ones = np.ones((P, 1), bf)
    bias_t = np.ascontiguousarray(
        np.stack([tr(bq), tr(bk), tr(bo_eff)], axis=1)
    )

    maps = []
    for core in range(8):
        b, p = core // 2, core % 2
        lo, hi = (0, 3) if p == 0 else (1, 2)
        # [P, chunk, ci, t'] = x[b, chunk*BLK+t', ci*P+p]
        xTv = np.ascontiguousarray(
            x[b].reshape(4, BLK, NT, P).transpose(3, 0, 2, 1)
        ).astype(bf)
        xvb = np.ascontiguousarray(xTv[:, [2 * p, 2 * p + 1]])
        xqb = np.ascontiguousarray(xTv[:, [lo, hi]])
        maps.append(
            {
                "xT": xTv,
                "xv": xvb,
                "xq": xqb,
                "Wk": Wk_t,
                "Wq": Wq_t,
                "Wv": Wv_t,
                "Wo": Wo_t,
                "bias_t": bias_t,
                "masks_t": np.ascontiguousarray(np.concatenate(
                    [
                        ones,
                        mask(CA, lo * BLK, CA + BLK),
                        mask(CB + njA * P, hi * BLK, CB + BLK),
                    ],
                    axis=1,
                )),
            }
        )
    return maps


def gather_output(results, T=T_, C=C_, B=B_):
    BLK = T // 4
    out = np.empty((B, T, C), np.float32)
    for core in range(8):
        b, p = core // 2, core % 2
        lo, hi = (0, 3) if p == 0 else (1, 2)
        oT = results[core]["outT"]
        out[b, lo * BLK:(lo + 1) * BLK] = oT[:, 0:BLK].T
        out[b, hi * BLK:(hi + 1) * BLK] = oT[:, BLK:2 * BLK].T
    return out


_NC_CACHE = {}


def kernel(x, Wq, bq, Wk, bk, Wv, bv, Wo, bo):
    from concourse.bass_utils import run_bass_kernel_spmd

    key = "full"
    if key not in _NC_CACHE:
        _NC_CACHE[key] = build_program()
    nc = _NC_CACHE[key]
    in_maps = make_core_inputs(x, Wq, bq, Wk, bk, Wv, bv, Wo, bo)
    res = run_bass_kernel_spmd(nc, in_maps, list(range(8))).results
    return gather_output(res)


# revision 42
# speedup vs baseline: 1.1588x; 1.1588x over previous
"""Causal full-d_model attention (B=4, T=2048, C=1024) on 8 Trainium2 cores.

Sharding: core = 2*b + p handles batch b and two 512-row sequence blocks,
p=0 -> blocks {0, 3}, p=1 -> blocks {1, 2} (pairing balances causal work).
K/V projections are split across the pair: each core projects its own
1024-token half (per-core input data places the right half in the same
program slot), then an intra-pair AllGather ([[0,1],[2,3],[4,5],[6,7]] -
the valid LNC1_4x2 shape) assembles the full kT / v, read back into SBUF
in global token order.  This halves the projection matmul work vs
computing K/V redundantly on both cores.

All matmul operands are bf16 (same PE rate as f32r, but FWL halves
LDWEIGHTS and every DMA byte count halves).  Accumulation is fp32 in
PSUM; softmax denominator / reciprocal / biases stay fp32.

On-device layout is transposed ([feature, token]) so every matmul
contracts along the partition axis:
    kT/qT = W.T @ xT            (projection)
    v     = xT.T @ Wv           (token-partition layout)
    scoresT[j, i] = kT_j.T @ qT (j on partitions)
    attnT[c, i]  += v_j.T @ probsT
    outT          = Wo_slice.T @ attnT
Softmax is unnormalized (scores ~ N(0,1), exp is safe); the denominator
comes from an M=1 ones-column matmul over masked exp tiles and is applied
at the PSUM->SBUF copy via a DRAM-broadcast reciprocal row.  Causal masks
arrive as per-core input data, so all 8 cores run one SPMD program.

Input loads ride the sync HWDGE FIFO in exact need-order (first matmul
needs only ~1.3 MB); the AllGather staging/readback path uses the gpsimd
queue so it never blocks input streaming.  Phase-2 emission order hides
the reciprocal round-trip (scores-B between PV-A and O-A) and PV
accumulates in two 4-bank PSUM groups to fit the 8 banks.
"""

import math

import numpy as np

P = 128          # SBUF partitions
B_, T_, C_ = 4, 2048, 1024
RG = [[0, 1], [2, 3], [4, 5], [6, 7]]   # intra-pair replica groups


def _emit(nc, tc, aps, T, C):
    import concourse.bass as bass
    from concourse import mybir
    from concourse.tile_rust import add_dep_helper
    from contextlib import ExitStack

    AFT = mybir.ActivationFunctionType
    f32 = mybir.dt.float32
    bf16 = mybir.dt.bfloat16

    NT = C // P            # feature tiles (8)
    BLK = T // 4           # sequence block = i-slot width FB (512)
    TL = 2 * BLK           # local query tokens per core
    TH = T // 2            # K/V tokens projected per core
    FB = BLK               # matmul moving free dim
    NCHL = TH // BLK       # local K/V chunks (2)
    njA = (2 * BLK) // P   # 8  j-tiles for slot A
    njB = (4 * BLK) // P   # 16 j-tiles for slot B
    CA = P * (njA - 1)
    NJB0 = njA             # slot-B j-tiles < NJB0 are causally all-ones
    CB = P * (njB - 1 - NJB0)
    SC = 1.0 / math.sqrt(C)

    (xT, xq, Wk, Wq, Wv, Wo, bias_t, masks_t, rec_dram, outT) = aps

    with ExitStack() as ctx:
        singles = ctx.enter_context(tc.tile_pool(name="singles", bufs=1))
        kpool = ctx.enter_context(tc.tile_pool(name="kpool", bufs=1))
        qpool = ctx.enter_context(tc.tile_pool(name="qpool", bufs=1))
        vpool = ctx.enter_context(tc.tile_pool(name="vpool", bufs=1))
        wopool = ctx.enter_context(tc.tile_pool(name="wopool", bufs=1))
        psp = ctx.enter_context(tc.tile_pool(name="psp", bufs=8, space="PSUM"))
        dramp = ctx.enter_context(tc.tile_pool(name="dramp", bufs=1,
                                               space="DRAM"))

        # bias3 packs bq/bk/bo; m_all packs ones + both mask masters, so
        # the startup FIFO spends its ~0.6us-per-DMA dispatch budget on
        # the critical wk/xc loads instead of four tiny transfers.
        bias3 = singles.tile([P, 3, NT], f32, name="bias3")
        m_all = singles.tile([P, 1 + (CA + FB) + (CB + FB)], bf16,
                             name="m_all")
        bq_sb, bk_sb, bo_sb = bias3[:, 0], bias3[:, 1], bias3[:, 2]
        ones_sb = m_all[:, 0:1]
        mA_sb = m_all[:, 1:1 + CA + FB]
        mB_sb = m_all[:, 1 + CA + FB:]

        kT_sb = kpool.tile([P, NT, T], bf16, name="kT_sb")
        qT_sb = qpool.tile([P, NT, TL], bf16, name="qT_sb")
        v_sb = vpool.tile([P, T // P, C], bf16, name="v_sb")
        wo_sb = wopool.tile([P, NT, NT, P], bf16, name="wo_sb")

        # DRAM bounce buffers for the intra-pair AllGather
        kag_in = dramp.tile([P, NT, TH], bf16, name="kag_in")
        kag_out = dramp.tile([2, P, NT, TH], bf16, name="kag_out")
        vag_in = dramp.tile([P, TH // P, C], bf16, name="vag_in")
        vag_out = dramp.tile([2, P, TH // P, C], bf16, name="vag_out")

        # ------- phase 1: K-half, V-half (+AllGathers), then Q -------
        with ExitStack() as p1:
            wpool = p1.enter_context(tc.tile_pool(name="wpool", bufs=1))
            xcpool = p1.enter_context(tc.tile_pool(name="xcpool", bufs=1))
            xqpool = p1.enter_context(tc.tile_pool(name="xqpool", bufs=1))
            kstp = p1.enter_context(tc.tile_pool(name="kstp", bufs=2))
            vstp = p1.enter_context(tc.tile_pool(name="vstp", bufs=6))

            wk_sb = wpool.tile([P, NT, NT, P], bf16, name="wk_sb")
            wq_sb = wpool.tile([P, NT, NT, P], bf16, name="wq_sb")
            wv_sb = wpool.tile([P, NT, C], bf16, name="wv_sb")
            xc_sb = xcpool.tile([P, NCHL, NT, BLK], bf16, name="xc_sb")
            xq_sb = xqpool.tile([P, 2, NT, FB], bf16, name="xq_sb")

            # input loads on the sync FIFO in exact need-order
            nc.sync.dma_start(out=wk_sb[:, 0], in_=Wk[:, 0])
            nc.sync.dma_start(out=xc_sb[:, 0, :NT // 2], in_=xT[:, 0, :NT // 2])
            nc.sync.dma_start(out=xc_sb[:, 0, NT // 2:], in_=xT[:, 0, NT // 2:])
            nc.sync.dma_start(out=bias3, in_=bias_t)
            for co in range(1, NT):
                nc.sync.dma_start(out=wk_sb[:, co], in_=Wk[:, co])
            nc.sync.dma_start(out=xc_sb[:, 1, :NT // 2], in_=xT[:, 1, :NT // 2])
            nc.sync.dma_start(out=xc_sb[:, 1, NT // 2:], in_=xT[:, 1, NT // 2:])
            nc.sync.dma_start(out=wv_sb, in_=Wv)
            nc.sync.dma_start(out=xq_sb, in_=xq)
            for co in range(NT):
                nc.sync.dma_start(out=wq_sb[:, co], in_=Wq[:, co])
            nc.sync.dma_start(out=m_all, in_=masks_t)
            nc.sync.dma_start(out=wo_sb, in_=Wo)

            # HAM warm-up: ~3.5us of tiny matmuls on the early-arriving
            # bias tensor while the first real operands are still in
            # flight, so the PE clock gate is already at 8/8 (2.4 GHz)
            # when the K matmuls start (saves the ~1.7us cold-start).
            ps_w = psp.tile([P, BLK], f32, name="ps_w", tag="ps")
            for i in range(56):
                nc.tensor.matmul(
                    ps_w[0:8, 0:8],
                    bias3[:, 0, :],
                    bias3[:, 0, :],
                    start=True,
                    stop=True,
                    skip_group_check=True,
                )

            # K-half: kT_loc = Wk.T @ x_half (+bk), staged to DRAM per
            # panel so the AllGather can trigger as early as possible
            for l in range(NCHL):
                kst = kstp.tile([P, NT, BLK], bf16, name="kst", tag="kst")
                for co in range(NT):
                    ps = psp.tile([P, BLK], f32, name="ps_k", tag="ps")
                    for ci in range(NT):
                        nc.tensor.matmul(
                            ps,
                            wk_sb[:, co, ci, :],
                            xc_sb[:, l, ci, :],
                            start=(ci == 0),
                            stop=(ci == NT - 1),
                        )
                    nc.scalar.activation(
                        out=kst[:, co, :], in_=ps, func=AFT.Identity,
                        bias=bk_sb[:, co:co + 1],
                    )
                nc.gpsimd.dma_start(
                    out=kag_in[:, :, l * BLK:(l + 1) * BLK], in_=kst
                )
            nc.gpsimd.collective_compute(
                "AllGather",
                mybir.AluOpType.bypass,
                replica_groups=RG,
                ins=[kag_in.opt()],
                outs=[kag_out.opt()],
            )

            # V-half: v_loc = x_half @ Wv, staged to DRAM
            # (bv is folded into bo_t on the host)
            for l in range(NCHL):
                for jt in range(BLK // P):
                    for ch in range(C // 512):
                        ps = psp.tile([P, 512], f32, name="ps_v", tag="ps")
                        for ci in range(NT):
                            nc.tensor.matmul(
                                ps,
                                xc_sb[:, l, ci, jt * P:(jt + 1) * P],
                                wv_sb[:, ci, ch * 512:(ch + 1) * 512],
                                start=(ci == 0),
                                stop=(ci == NT - 1),
                            )
                        vs = vstp.tile([P, 512], bf16, name="vs", tag="vs")
                        nc.vector.tensor_copy(vs, ps)
                        # on sync: drains independently of the collective
                        # triggers queued on gpsimd, so casts never starve
                        nc.sync.dma_start(
                            out=vag_in[:, l * (BLK // P) + jt,
                                       ch * 512:(ch + 1) * 512],
                            in_=vs,
                        )
            nc.gpsimd.collective_compute(
                "AllGather",
                mybir.AluOpType.bypass,
                replica_groups=RG,
                ins=[vag_in.opt()],
                outs=[vag_out.opt()],
            )

            # readbacks in global token order (h = rank in pair); on the
            # sync queue so they never delay the AllGather triggers, and
            # in 512-token pieces so scores/PV start consuming the first
            # j-tiles ~3us after AllGather-done instead of ~11us
            for h in range(2):
                for pc in range(NCHL):
                    nc.sync.dma_start(
                        out=kT_sb[:, :, h * TH + pc * BLK:
                                  h * TH + (pc + 1) * BLK],
                        in_=kag_out[h][:, :, pc * BLK:(pc + 1) * BLK],
                    )
            for h in range(2):
                for pc in range(NCHL):
                    nc.sync.dma_start(
                        out=v_sb[:, h * (TH // P) + pc * (BLK // P):
                                 h * (TH // P) + (pc + 1) * (BLK // P), :],
                        in_=vag_out[h][:, pc * (BLK // P):
                                       (pc + 1) * (BLK // P), :],
                    )

            # Q: qT = Wq.T @ xq (+bq) for the two local 512-blocks
            for s in range(2):
                for co in range(NT):
                    ps = psp.tile([P, FB], f32, name="ps_q", tag="ps")
                    for ci in range(NT):
                        nc.tensor.matmul(
                            ps,
                            wq_sb[:, co, ci, :],
                            xq_sb[:, s, ci, :],
                            start=(ci == 0),
                            stop=(ci == NT - 1),
                        )
                    nc.scalar.activation(
                        out=qT_sb[:, co, s * FB:(s + 1) * FB],
                        in_=ps,
                        func=AFT.Identity,
                        bias=bq_sb[:, co:co + 1],
                    )

        # -------- phase 2: attention + output projection --------
        with ExitStack() as p2:
            probsp = p2.enter_context(tc.tile_pool(name="probsp",
                                                   bufs=njA + njB))
            attnp = p2.enter_context(tc.tile_pool(name="attnp", bufs=2))
            recp = p2.enter_context(tc.tile_pool(name="recp", bufs=2))
            ostagep = p2.enter_context(tc.tile_pool(name="ostagep", bufs=2))

            SLOTS = [(njA, 0, CA, mA_sb), (njB, NJB0, CB, mB_sb)]

            def emit_scores(a):
                nj, j0m, Cm, m_sb = SLOTS[a]
                pjs = []
                ps_den = psp.tile([1, FB], f32, name="ps_den", tag="ps")
                for jt in range(nj):
                    ps_s = psp.tile([P, FB], f32, name="ps_s", tag="ps")
                    for ci in range(NT):
                        nc.tensor.matmul(
                            ps_s,
                            kT_sb[:, ci, jt * P:(jt + 1) * P],
                            qT_sb[:, ci, a * FB:(a + 1) * FB],
                            start=(ci == 0),
                            stop=(ci == NT - 1),
                        )
                    pj = probsp.tile([P, FB], bf16, name="pj", tag="pj")
                    nc.scalar.activation(out=pj, in_=ps_s, func=AFT.Exp, scale=SC)
                    if jt >= j0m:  # earlier j-tiles are all-ones on every core
                        s0 = Cm - P * (jt - j0m)
                        nc.vector.tensor_mul(pj, pj, m_sb[:, s0:s0 + FB])
                    nc.tensor.matmul(
                        ps_den,
                        ones_sb,
                        pj,
                        start=(jt == 0),
                        stop=(jt == nj - 1),
                        skip_group_check=True,
                    )
                    pjs.append(pj)
                # 1/denominator: quick copy releases the PSUM bank, then the
                # slow reciprocal runs off the SBUF copy; broadcast to 128
                # partitions via a stride-0 DRAM read.
                den_sb = recp.tile([1, FB], f32, name="den_sb", tag="den_sb")
                nc.scalar.copy(den_sb, ps_den)
                rrow = recp.tile([1, FB], f32, name="rrow", tag="rrow")
                nc.vector.reciprocal(rrow, den_sb)
                rec_w = nc.sync.dma_start(out=rec_dram[a:a + 1, :], in_=rrow)
                recipB = recp.tile([P, FB], f32, name="recipB", tag="recipB")
                rec_row = rec_dram[a, :]
                rec_bcast = bass.AP(
                    tensor=rec_row.tensor,
                    offset=rec_row.offset,
                    ap=[[0, P]] + [list(d) for d in rec_row.ap],
                )
                rec_r = nc.sync.dma_start(out=recipB, in_=rec_bcast)
                add_dep_helper(rec_r.ins, rec_w.ins, reason="rec_dram RAW")
                return pjs, recipB

            def emit_pv_group(a, pjs, recipB, attn_sb, g0):
                # one 4-bank PSUM accumulation group of PV; the attn
                # multiplies drain the banks while later PE work runs
                nj = SLOTS[a][0]
                ps_attn = [
                    psp.tile([P, FB], f32, name="ps_attn", tag="ps")
                    for _ in range(NT // 2)
                ]
                for jt in range(nj):
                    for k, ct in enumerate(range(g0, g0 + NT // 2)):
                        nc.tensor.matmul(
                            ps_attn[k],
                            v_sb[:, jt, ct * P:(ct + 1) * P],
                            pjs[jt],
                            start=(jt == 0),
                            stop=(jt == nj - 1),
                            skip_group_check=True,
                        )
                for k, ct in enumerate(range(g0, g0 + NT // 2)):
                    nc.vector.tensor_mul(attn_sb[:, ct, :], ps_attn[k], recipB)

            def emit_oproj(a, attn_sb):
                for co in range(NT):
                    ps_o = psp.tile([P, FB], f32, name="ps_o", tag="ps")
                    for ci in range(NT):
                        nc.tensor.matmul(
                            ps_o,
                            wo_sb[:, co, ci, :],
                            attn_sb[:, ci, :],
                            start=(ci == 0),
                            stop=(ci == NT - 1),
                        )
                    os_ = ostagep.tile([P, FB], f32, name="os_", tag="os")
                    nc.scalar.activation(
                        out=os_, in_=ps_o, func=AFT.Identity,
                        bias=bo_sb[:, co:co + 1],
                    )
                    nc.sync.dma_start(
                        out=outT[co * P:(co + 1) * P, a * FB:(a + 1) * FB],
                        in_=os_,
                    )

            # Both score passes run before any PV: scores only need kT
            # (the first AllGather), so the v AllGather + readback gets
            # ~28us more compute to hide under before PV-A reads v.
            # PV-B group 1 sits between PV-A and O-A so the attn-multiply
            # latency of each slot hides under the other's PE work.
            pjs_A, recB_A = emit_scores(0)
            pjs_B, recB_B = emit_scores(1)
            attn_A = attnp.tile([P, NT, FB], bf16, name="attn_A", tag="attn")
            attn_B = attnp.tile([P, NT, FB], bf16, name="attn_B", tag="attn")
            emit_pv_group(0, pjs_A, recB_A, attn_A, 0)
            emit_pv_group(0, pjs_A, recB_A, attn_A, NT // 2)
            emit_pv_group(1, pjs_B, recB_B, attn_B, 0)
            emit_oproj(0, attn_A)
            emit_pv_group(1, pjs_B, recB_B, attn_B, NT // 2)
            emit_oproj(1, attn_B)


def build_program(T=T_, C=C_, num_cores=8):
    """Build and compile the SPMD Bass program."""
    from concourse import bacc, mybir
    import concourse.tile as tile

    f32 = mybir.dt.float32
    bf16 = mybir.dt.bfloat16
    NT = C // P
    BLK = T // 4
    TL = 2 * BLK
    njA = (2 * BLK) // P
    njB = (4 * BLK) // P
    CA = P * (njA - 1)
    CB = P * (njB - 1 - njA)

    nc = bacc.Bacc(
        "TRN2", target_bir_lowering=False, debug=False, num_devices=num_cores
    )
    xT = nc.dram_tensor("xT", [P, 2, NT, BLK], bf16, kind="ExternalInput").ap()
    xq = nc.dram_tensor("xq", [P, 2, NT, BLK], bf16, kind="ExternalInput").ap()
    Wk = nc.dram_tensor("Wk", [P, NT, NT, P], bf16, kind="ExternalInput").ap()
    Wq = nc.dram_tensor("Wq", [P, NT, NT, P], bf16, kind="ExternalInput").ap()
    Wv = nc.dram_tensor("Wv", [P, NT, C], bf16, kind="ExternalInput").ap()
    Wo = nc.dram_tensor("Wo", [P, NT, NT, P], bf16, kind="ExternalInput").ap()
    bias_t = nc.dram_tensor("bias_t", [P, 3, NT], f32,
                            kind="ExternalInput").ap()
    masks_t = nc.dram_tensor(
        "masks_t", [P, 1 + (CA + BLK) + (CB + BLK)], bf16,
        kind="ExternalInput",
    ).ap()
    rec_dram = nc.dram_tensor("rec_int", [2, BLK], f32).ap()
    outT = nc.dram_tensor("outT", [C, TL], f32, kind="ExternalOutput").ap()

    aps = (xT, xq, Wk, Wq, Wv, Wo, bias_t, masks_t, rec_dram, outT)
    with tile.TileContext(nc) as tc:
        _emit(nc, tc, aps, T, C)
    nc.compile()
    return nc


def make_core_inputs(x, Wq, bq, Wk, bk, Wv, bv, Wo, bo, T=T_, C=C_):
    """Per-core input maps (list of 8 dicts) for the SPMD program."""
    import ml_dtypes

    f = np.float32
    bf = ml_dtypes.bfloat16
    NT = C // P
    BLK = T // 4
    njA = (2 * BLK) // P
    njB = (4 * BLK) // P
    CA = P * (njA - 1)
    CB = P * (njB - 1 - njA)

    x = np.asarray(x, f)
    Wq, Wk, Wv, Wo = (np.asarray(w, f) for w in (Wq, Wk, Wv, Wo))
    bq, bk, bv, bo = (np.asarray(b, f) for b in (bq, bk, bv, bo))

    def panels(W):  # [C, C] -> [P, co, ci, m]: W[ci*P+p, co*P+m]
        return np.ascontiguousarray(
            W.reshape(NT, P, NT, P).transpose(1, 2, 0, 3)
        ).astype(bf)

    Wk_t = panels(Wk)
    Wq_t = panels(Wq)
    Wo_t = panels(Wo)
    # [C, C] -> [P, ci, m]: Wv[ci*P+p, m]
    Wv_t = np.ascontiguousarray(
        Wv.reshape(NT, P, C).transpose(1, 0, 2)
    ).astype(bf)
    bo_eff = (bv @ Wo + bo).astype(f)

    def tr(b):  # [C] -> [P, NT] with b_t[p, t] = b[t*P + p]
        return np.ascontiguousarray(b.reshape(NT, P).T)

    def mask(CC, i0, width):
        pp = np.arange(P, dtype=np.int64)[:, None]
        gg = np.arange(width, dtype=np.int64)[None, :]
        return (pp <= gg - CC + i0).astype(bf)

    ones = np.ones((P, 1), bf)
    bias_t = np.ascontiguousarray(
        np.stack([tr(bq), tr(bk), tr(bo_eff)], axis=1)
    )

    maps = []
    for core in range(8):
        b, p = core // 2, core % 2
        lo, hi = (0, 3) if p == 0 else (1, 2)
        # [P, chunk, ci, t'] = x[b, chunk*BLK+t', ci*P+p]
        xTv = np.ascontiguousarray(
            x[b].reshape(4, BLK, NT, P).transpose(3, 0, 2, 1)
        ).astype(bf)
        xhalf = np.ascontiguousarray(xTv[:, [2 * p, 2 * p + 1]])
        xqb = np.ascontiguousarray(xTv[:, [lo, hi]])
        maps.append(
            {
                "xT": xhalf,
                "xq": xqb,
                "Wk": Wk_t,
                "Wq": Wq_t,
                "Wv": Wv_t,
                "Wo": Wo_t,
                "bias_t": bias_t,
                "masks_t": np.ascontiguousarray(np.concatenate(
                    [
                        ones,
                        mask(CA, lo * BLK, CA + BLK),
                        mask(CB + njA * P, hi * BLK, CB + BLK),
                    ],
                    axis=1,
                )),
            }
        )
    return maps


def gather_output(results, T=T_, C=C_, B=B_):
    BLK = T // 4
    out = np.empty((B, T, C), np.float32)
    for core in range(8):
        b, p = core // 2, core % 2
        lo, hi = (0, 3) if p == 0 else (1, 2)
        oT = results[core]["outT"]
        out[b, lo * BLK:(lo + 1) * BLK] = oT[:, 0:BLK].T
        out[b, hi * BLK:(hi + 1) * BLK] = oT[:, BLK:2 * BLK].T
    return out


_NC_CACHE = {}


def kernel(x, Wq, bq, Wk, bk, Wv, bv, Wo, bo):
    from concourse.bass_utils import run_bass_kernel_spmd

    key = "full"
    if key not in _NC_CACHE:
        _NC_CACHE[key] = build_program()
    nc = _NC_CACHE[key]
    in_maps = make_core_inputs(x, Wq, bq, Wk, bk, Wv, bv, Wo, bo)
    res = run_bass_kernel_spmd(nc, in_maps, list(range(8))).results
    return gather_output(res)
